# revision 1
# baseline (speedup 1.0000x reference)
"""Locally-connected graph-conv kernel for Trainium2 (Bass/Tile).

Computes out[b,t,m] = sum_n x[b,t,n] * (S*W)[n,m] + bias[m] for
x [64, 2048, 208], W/S [208, 208], bias [208].

The ring-graph support S is a +-4 band (mod 208), so each half of the
output nodes only needs a 112-row slice of the contraction dim:
  block 0 (m 0..103):   n in {204..207} ++ {0..107}
  block 1 (m 104..207): n in {100..207} ++ {0..3}
Each output block is then a SINGLE [112,104] x [112,512] fp32 matmul with
the masked-weight block stationary in the PE array and x^T streaming as
the moving operand in 512-column blocks (long streams hide the fp32
LDWEIGHTS). The bias is fused into the PSUM->SBUF eviction as a
per-partition tensor_scalar add on VectorE.

Data-parallel over 8 NeuronCores: each core gets 16384 rows of the
flattened x, host-pre-assembled into a [224, 16384] tensor (two 112-row
halo blocks). DMA partition counts are multiples of 16 (the fast HWDGE
path: ~250 GB/s/instr vs ~27 otherwise); stores are [112]-row DMAs into
a [224, SHARD] output (8 pad rows per block, dropped at host gather).
x loads issue on the Sync HWDGE ring, stores on the Scalar ring, one-time
weight/bias setup on the GpSimd SWDGE queue so it never delays them.
The host transposes y^T back at gather.
"""

import numpy as np
from contextlib import ExitStack

import concourse.bacc as bacc
import concourse.mybir as mybir
import concourse.tile as tile
from concourse.bass_utils import run_bass_kernel_spmd

N = 208                      # nodes
HALF = 104                   # output nodes per block
K = 4                        # band half-width of S
NH = 2 * K + HALF            # 112 contraction rows per block (halo incl.)
NP = 112                     # padded store rows (multiple of 16)
N_CORES = 8
B, T = 64, 2048
ROWS_TOTAL = B * T           # 131072
SHARD = ROWS_TOTAL // N_CORES    # 16384 rows per core
TB = 512                     # moving-block columns per matmul (fp32 PSUM max)
TB2 = 2 * TB                 # eviction group (2 PSUM banks)
TOUT = 2048                  # t-columns per DMA chunk (~0.9 MB loads)
N_CHUNKS = SHARD // TOUT     # 8
SUB = TOUT // TB2            # 2 psum groups per chunk

FP32 = mybir.dt.float32

# halo row order (indices into the [208] node dim) for each block
ROWS0 = list(range(N - K, N)) + list(range(0, HALF + K))          # 112
ROWS1 = list(range(HALF - K, N)) + list(range(0, K))              # 112

_CACHE = {}
LAST_RESULTS = None          # BassKernelResults of the most recent run


def _kernel_body(tc):
    nc = tc.nc
    # rows 0:112 block0 halo, 112:224 block1 halo
    x_d = nc.dram_tensor("xh", [2 * NH, SHARD], FP32, kind="ExternalInput").ap()
    w_d = nc.dram_tensor("w", [N, N], FP32, kind="ExternalInput").ap()
    s_d = nc.dram_tensor("s", [N, N], FP32, kind="ExternalInput").ap()
    b_d = nc.dram_tensor("bias", [1, N], FP32, kind="ExternalInput").ap()
    o_d = nc.dram_tensor("outt", [2 * NP, SHARD], FP32, kind="ExternalOutput").ap()

    with ExitStack() as ctx:
        const = ctx.enter_context(tc.tile_pool(name="const", bufs=1))

        # One-time setup: w/s pieces on the Scalar HWDGE ring (fast issue,
        # idle at startup), bias on GpSimd. Stationary blocks wh0/wh1
        # [112, 104]: masked weight rows in halo order. Bias [104, 1].
        w0 = const.tile([NH, HALF], FP32, tag="w0")
        s0 = const.tile([NH, HALF], FP32, tag="s0")
        nc.scalar.dma_start(w0[0:K, :], w_d[N - K : N, 0:HALF])
        nc.scalar.dma_start(w0[K:NH, :], w_d[0 : HALF + K, 0:HALF])
        nc.scalar.dma_start(s0[0:K, :], s_d[N - K : N, 0:HALF])
        nc.scalar.dma_start(s0[K:NH, :], s_d[0 : HALF + K, 0:HALF])
        wh0 = const.tile([NH, HALF], FP32, tag="wh0")
        nc.vector.tensor_mul(wh0, w0, s0)
        w1 = const.tile([NH, HALF], FP32, tag="w1")
        s1 = const.tile([NH, HALF], FP32, tag="s1")
        nc.scalar.dma_start(w1[0 : HALF + K, :], w_d[HALF - K : N, HALF:N])
        nc.scalar.dma_start(w1[HALF + K : NH, :], w_d[0:K, HALF:N])
        nc.scalar.dma_start(s1[0 : HALF + K, :], s_d[HALF - K : N, HALF:N])
        nc.scalar.dma_start(s1[HALF + K : NH, :], s_d[0:K, HALF:N])
        wh1 = const.tile([NH, HALF], FP32, tag="wh1")
        nc.vector.tensor_mul(wh1, w1, s1)
        bA = const.tile([HALF, 1], FP32, tag="bA")
        bB = const.tile([HALF, 1], FP32, tag="bB")
        b_col = b_d.rearrange("o n -> n o")
        nc.gpsimd.dma_start(bA, b_col[0:HALF, :])
        nc.gpsimd.dma_start(bB, b_col[HALF:N, :])

        x0p = ctx.enter_context(tc.tile_pool(name="x0p", bufs=6))
        x1p = ctx.enter_context(tc.tile_pool(name="x1p", bufs=6))
        o0p = ctx.enter_context(tc.tile_pool(name="o0p", bufs=4))
        o1p = ctx.enter_context(tc.tile_pool(name="o1p", bufs=4))
        ps0p = ctx.enter_context(tc.tile_pool(name="ps0p", bufs=2, space="PSUM"))
        ps1p = ctx.enter_context(tc.tile_pool(name="ps1p", bufs=2, space="PSUM"))

        for c in range(N_CHUNKS):
            tsl = slice(c * TOUT, (c + 1) * TOUT)
            xh0 = x0p.tile([NH, TOUT], FP32, tag="xh0")
            xh1 = x1p.tile([NH, TOUT], FP32, tag="xh1")
            if c == 0:
                # split the critical-path first loads for 2x DMA concurrency
                nc.sync.dma_start(xh0[0:64, :], x_d[0:64, tsl])
                nc.sync.dma_start(xh0[64:NH, :], x_d[64:NH, tsl])
                nc.sync.dma_start(xh1[0:64, :], x_d[NH : NH + 64, tsl])
                nc.sync.dma_start(xh1[64:NH, :], x_d[NH + 64 : 2 * NH, tsl])
            else:
                nc.sync.dma_start(xh0, x_d[0:NH, tsl])
                nc.sync.dma_start(xh1, x_d[NH : 2 * NH, tsl])

            o0_t = o0p.tile([NP, TOUT], FP32, tag="o0")
            o1_t = o1p.tile([NP, TOUT], FP32, tag="o1")
            for s in range(SUB):
                g = slice(s * TB2, (s + 1) * TB2)
                ga = slice(s * TB2, s * TB2 + TB)
                gb = slice(s * TB2 + TB, (s + 1) * TB2)
                # [104, 1024] PSUM tiles (2 banks); each matmul fills one bank
                ps0 = ps0p.tile([HALF, TB2], FP32, tag="ps0")
                nc.tensor.matmul(ps0[:, 0:TB], wh0, xh0[:, ga], start=True, stop=True)
                nc.tensor.matmul(ps0[:, TB:TB2], wh0, xh0[:, gb], start=True, stop=True)
                ps1 = ps1p.tile([HALF, TB2], FP32, tag="ps1")
                nc.tensor.matmul(ps1[:, 0:TB], wh1, xh1[:, ga], start=True, stop=True)
                nc.tensor.matmul(ps1[:, TB:TB2], wh1, xh1[:, gb], start=True, stop=True)
                # eviction + per-partition bias on VectorE
                nc.vector.tensor_scalar_add(o0_t[0:HALF, g], ps0, bA)
                nc.vector.tensor_scalar_add(o1_t[0:HALF, g], ps1, bB)
            # per-chunk stores (112 rows, 8 pad) on the Scalar HWDGE ring;
            # the last chunk's second-block store rides the by-then-idle Sync
            # ring so the two tail stores run in parallel
            nc.scalar.dma_start(o_d[0:NP, tsl], o0_t)
            if c == N_CHUNKS - 1:
                nc.sync.dma_start(o_d[NP : 2 * NP, tsl], o1_t)
            else:
                nc.scalar.dma_start(o_d[NP : 2 * NP, tsl], o1_t)


def _build():
    nc = bacc.Bacc(
        "TRN2",
        target_bir_lowering=False,
        debug=False,
        num_devices=N_CORES,
    )
    with tile.TileContext(nc) as tc:
        _kernel_body(tc)
    nc.compile()
    return nc


def kernel(x, W, b, S):
    global LAST_RESULTS
    nc = _CACHE.get("nc")
    if nc is None:
        nc = _build()
        _CACHE["nc"] = nc

    xf = np.asarray(x, np.float32).reshape(ROWS_TOTAL, N)
    Wf = np.ascontiguousarray(np.asarray(W, np.float32))
    Sf = np.ascontiguousarray(np.asarray(S, np.float32))
    bf = np.ascontiguousarray(np.asarray(b, np.float32).reshape(1, N))

    in_maps = []
    for i in range(N_CORES):
        xt = xf[i * SHARD : (i + 1) * SHARD].T          # [208, SHARD] view
        xh = np.empty((2 * NH, SHARD), np.float32)
        xh[0:NH] = xt[ROWS0]
        xh[NH : 2 * NH] = xt[ROWS1]
        in_maps.append({"xh": xh, "w": Wf, "s": Sf, "bias": bf})
    res = run_bass_kernel_spmd(nc, in_maps, core_ids=list(range(N_CORES)))
    LAST_RESULTS = res
    out = np.empty((ROWS_TOTAL, N), np.float32)
    for i, r in enumerate(res.results):
        yt = r["outt"]                                  # [224, SHARD]
        out[i * SHARD : (i + 1) * SHARD, 0:HALF] = yt[0:HALF].T
        out[i * SHARD : (i + 1) * SHARD, HALF:N] = yt[NP : NP + HALF].T
    return out.reshape(B, T, N)



# revision 2
# speedup vs baseline: 1.2949x; 1.2949x over previous
"""Locally-connected graph-conv kernel for Trainium2 (Bass/Tile).

Computes out[b,t,m] = sum_n x[b,t,n] * (S*W)[n,m] + bias[m] for
x [64, 2048, 208], W/S [208, 208], bias [208].

The ring-graph support S is a +-4 band (mod 208), so each half of the
output nodes only needs a 112-row slice of the contraction dim. With a
rotated node layout (row j holds node (j-4) mod 208, 216 rows total):
  block 0 (m 0..103):   rotated rows   0..111
  block 1 (m 104..207): rotated rows 104..215
Each output block is a SINGLE [112,104] x [112,512] matmul with the
host-premasked weight block stationary in the PE array and x^T streaming
as the moving operand. Bias is fused into the PSUM->SBUF eviction as a
per-partition tensor_scalar add on VectorE.

Everything that touches HBM is bf16 (PSUM accumulation stays fp32):
the 2e-2 rel-err budget dwarfs bf16 rounding (~5e-3), and it halves DMA
bytes, quarters PE time, and halves DVE time vs fp32.

Data-parallel over 8 NeuronCores: each core gets 16384 rows of the
flattened x, host-pre-assembled into the rotated [216, 16384] bf16
tensor. DMA partition counts are multiples of 16 (the fast HWDGE path:
~250 GB/s/instr vs ~27 otherwise). Output blocks store into an
overlapping [216, SHARD] layout on the SAME queue (block0 rows 0:112
then block1 rows 104:216; FIFO makes block1's valid rows 104..111 land
last). x loads ride the Sync HWDGE ring, stores the Scalar ring; the
first two chunks' loads split across both rings (stores haven't started
yet), one-time weight/bias setup goes first on Scalar/GpSimd.
The host transposes y^T back at gather.
"""

import numpy as np
import ml_dtypes
from contextlib import ExitStack

import concourse.bacc as bacc
import concourse.mybir as mybir
import concourse.tile as tile
from concourse.bass_utils import run_bass_kernel_spmd

N = 208                      # nodes
HALF = 104                   # output nodes per block
K = 4                        # band half-width of S
NH = 2 * K + HALF            # 112 contraction rows per block (halo incl.)
NR = N + 2 * K               # 216 rotated rows
N_CORES = 8
B, T = 64, 2048
ROWS_TOTAL = B * T           # 131072
SHARD = ROWS_TOTAL // N_CORES    # 16384 rows per core
TB = 512                     # moving-block columns per matmul (fp32 PSUM max)
TB2 = 2 * TB                 # eviction group (2 PSUM banks)
TOUT = 2048                  # t-columns per DMA chunk (~0.45 MB bf16 loads)
N_CHUNKS = SHARD // TOUT     # 8
SUB = TOUT // TB2            # 2 psum groups per chunk

FP32 = mybir.dt.float32
BF16 = mybir.dt.bfloat16
NP_BF16 = ml_dtypes.bfloat16

# halo row order (indices into the [208] node dim) for each block
ROWS0 = list(range(N - K, N)) + list(range(0, HALF + K))          # 112
ROWS1 = list(range(HALF - K, N)) + list(range(0, K))              # 112

_CACHE = {}
LAST_RESULTS = None          # BassKernelResults of the most recent run


def _kernel_body(tc):
    nc = tc.nc
    # rotated x: row j = node (j-4) mod 208; block0 = rows 0:112,
    # block1 = rows 104:216
    x_d = nc.dram_tensor("xh", [NR, SHARD], BF16, kind="ExternalInput").ap()
    w_d = nc.dram_tensor("wh", [NH, N], BF16, kind="ExternalInput").ap()
    b_d = nc.dram_tensor("bias", [N, 1], FP32, kind="ExternalInput").ap()
    o_d = nc.dram_tensor("outt", [NR, SHARD], BF16, kind="ExternalOutput").ap()

    with ExitStack() as ctx:
        const = ctx.enter_context(tc.tile_pool(name="const", bufs=1))

        # One-time setup: host-premasked halo-ordered weights, one DMA on
        # the Scalar HWDGE ring (idle at startup); bias halves on GpSimd.
        wh = const.tile([NH, N], BF16, tag="wh")
        nc.scalar.dma_start(wh, w_d)
        bA = const.tile([HALF, 1], FP32, tag="bA")
        bB = const.tile([HALF, 1], FP32, tag="bB")
        nc.gpsimd.dma_start(bA, b_d[0:HALF, :])
        nc.gpsimd.dma_start(bB, b_d[HALF:N, :])

        x0p = ctx.enter_context(tc.tile_pool(name="x0p", bufs=6))
        x1p = ctx.enter_context(tc.tile_pool(name="x1p", bufs=6))
        o0p = ctx.enter_context(tc.tile_pool(name="o0p", bufs=4))
        o1p = ctx.enter_context(tc.tile_pool(name="o1p", bufs=4))
        ps0p = ctx.enter_context(tc.tile_pool(name="ps0p", bufs=2, space="PSUM"))
        ps1p = ctx.enter_context(tc.tile_pool(name="ps1p", bufs=2, space="PSUM"))

        for c in range(N_CHUNKS):
            tsl = slice(c * TOUT, (c + 1) * TOUT)
            xh0 = x0p.tile([NH, TOUT], BF16, tag="xh0")
            xh1 = x1p.tile([NH, TOUT], BF16, tag="xh1")
            if c < 2:
                # stores haven't started: split loads across both rings
                nc.sync.dma_start(xh0[0:64, :], x_d[0:64, tsl])
                nc.scalar.dma_start(xh0[64:NH, :], x_d[64:NH, tsl])
                nc.sync.dma_start(xh1[0:64, :], x_d[HALF : HALF + 64, tsl])
                nc.scalar.dma_start(xh1[64:NH, :], x_d[HALF + 64 : NR, tsl])
            else:
                nc.sync.dma_start(xh0, x_d[0:NH, tsl])
                nc.sync.dma_start(xh1, x_d[HALF:NR, tsl])

            o0_t = o0p.tile([NH, TOUT], BF16, tag="o0")
            o1_t = o1p.tile([NH, TOUT], BF16, tag="o1")
            for s in range(SUB):
                g = slice(s * TB2, (s + 1) * TB2)
                ga = slice(s * TB2, s * TB2 + TB)
                gb = slice(s * TB2 + TB, (s + 1) * TB2)
                # [104, 1024] PSUM tiles (2 banks); each matmul fills one bank
                ps0 = ps0p.tile([HALF, TB2], FP32, tag="ps0")
                nc.tensor.matmul(ps0[:, 0:TB], wh[:, 0:HALF], xh0[:, ga], start=True, stop=True)
                nc.tensor.matmul(ps0[:, TB:TB2], wh[:, 0:HALF], xh0[:, gb], start=True, stop=True)
                ps1 = ps1p.tile([HALF, TB2], FP32, tag="ps1")
                nc.tensor.matmul(ps1[:, 0:TB], wh[:, HALF:N], xh1[:, ga], start=True, stop=True)
                nc.tensor.matmul(ps1[:, TB:TB2], wh[:, HALF:N], xh1[:, gb], start=True, stop=True)
                # eviction + per-partition bias on VectorE, fp32 -> bf16
                nc.vector.tensor_scalar_add(o0_t[0:HALF, g], ps0, bA)
                nc.vector.tensor_scalar_add(o1_t[0:HALF, g], ps1, bB)
            # overlapping stores, same Scalar queue: block0 rows 0:112 first,
            # block1 rows 104:216 second so its valid rows 104..111 win
            nc.scalar.dma_start(o_d[0:NH, tsl], o0_t)
            nc.scalar.dma_start(o_d[HALF:NR, tsl], o1_t)


def _build():
    nc = bacc.Bacc(
        "TRN2",
        target_bir_lowering=False,
        debug=False,
        num_devices=N_CORES,
    )
    with tile.TileContext(nc) as tc:
        _kernel_body(tc)
    nc.compile()
    return nc


def kernel(x, W, b, S):
    global LAST_RESULTS
    nc = _CACHE.get("nc")
    if nc is None:
        nc = _build()
        _CACHE["nc"] = nc

    xf = np.asarray(x, np.float32).reshape(ROWS_TOTAL, N)
    SW = (np.asarray(S, np.float32) * np.asarray(W, np.float32))
    wh = np.empty((NH, N), NP_BF16)
    wh[:, 0:HALF] = SW[ROWS0, 0:HALF]
    wh[:, HALF:N] = SW[ROWS1, HALF:N]
    bf = np.ascontiguousarray(np.asarray(b, np.float32).reshape(N, 1))

    in_maps = []
    for i in range(N_CORES):
        xt = xf[i * SHARD : (i + 1) * SHARD].T          # [208, SHARD] view
        xh = np.empty((NR, SHARD), NP_BF16)
        xh[0:K] = xt[N - K : N]
        xh[K : N + K] = xt
        xh[N + K : NR] = xt[0:K]
        in_maps.append({"xh": xh, "wh": wh, "bias": bf})
    res = run_bass_kernel_spmd(nc, in_maps, core_ids=list(range(N_CORES)))
    LAST_RESULTS = res
    out = np.empty((ROWS_TOTAL, N), np.float32)
    for i, r in enumerate(res.results):
        yt = r["outt"]                                  # [216, SHARD] bf16
        out[i * SHARD : (i + 1) * SHARD, 0:HALF] = yt[0:HALF].T
        out[i * SHARD : (i + 1) * SHARD, HALF:N] = yt[HALF:N].T
    return out.reshape(B, T, N)


# revision 3
# speedup vs baseline: 1.4980x; 1.1568x over previous
"""Locally-connected graph-conv kernel for Trainium2 (Bass/Tile).

Computes out[b,t,m] = sum_n x[b,t,n] * (S*W)[n,m] + bias[m] for
x [64, 2048, 208], W/S [208, 208], bias [208].

The ring-graph support S is a +-4 band (mod 208), so each half of the
output nodes only needs a 112-row slice of the contraction dim. With a
rotated node layout (row j holds node (j-4) mod 208, 216 rows total):
  block 0 (m 0..103):   rotated rows   0..111
  block 1 (m 104..207): rotated rows 104..215
Each output block is a SINGLE [112,104] x [112,512] matmul with the
host-premasked weight block stationary in the PE array and x^T streaming
as the moving operand.

Everything that touches HBM is bf16 (PSUM accumulation stays fp32):
the 2e-2 rel-err budget dwarfs bf16 rounding (~5e-3), and it halves DMA
bytes vs fp32. HBM per NeuronCore is ~358 GB/s, so the 14.2 MB/core of
traffic floors at ~40 us; everything else is shaped to stay under that:
 - 917 KB DMA instructions ([112, 4096], 8 KB/partition descriptors)
   run at ~341 GB/s vs ~245 GB/s for 458 KB ones, and fewer DMAs mean
   less per-instruction issue/semaphore time on the queue engines.
 - PSUM->SBUF eviction is stuck at 1 elem/lane/cycle (fp32 PSUM source),
   so block 0 evicts on VectorE (tensor_scalar add, 0.96 GHz) and
   block 1 on ScalarE (Identity activation with bias AP, 1.2 GHz),
   halving the serial eviction chain.
Output blocks store into an overlapping [216, SHARD] layout on the SAME
queue (block0 rows 0:112 then block1 rows 104:216; FIFO makes block1's
valid rows 104..111 land last). x loads ride the Sync HWDGE ring, stores
the Scalar ring; the first chunk's loads split across both rings (stores
haven't started yet). The host transposes y^T back at gather.
"""

import numpy as np
import ml_dtypes
from contextlib import ExitStack

import concourse.bacc as bacc
import concourse.mybir as mybir
import concourse.tile as tile
from concourse.bass_utils import run_bass_kernel_spmd

N = 208                      # nodes
HALF = 104                   # output nodes per block
K = 4                        # band half-width of S
NH = 2 * K + HALF            # 112 contraction rows per block (halo incl.)
NR = N + 2 * K               # 216 rotated rows
N_CORES = 8
B, T = 64, 2048
ROWS_TOTAL = B * T           # 131072
SHARD = ROWS_TOTAL // N_CORES    # 16384 rows per core
TB = 512                     # moving-block columns per matmul (fp32 PSUM max)
TB2 = 2 * TB                 # eviction group (2 PSUM banks)
TOUT = 4096                  # t-columns per DMA chunk (~0.9 MB bf16 loads)
N_CHUNKS = SHARD // TOUT     # 4
SUB = TOUT // TB2            # 4 psum groups per chunk

FP32 = mybir.dt.float32
BF16 = mybir.dt.bfloat16
NP_BF16 = ml_dtypes.bfloat16
IDENT = mybir.ActivationFunctionType.Identity

# halo row order (indices into the [208] node dim) for each block
ROWS0 = list(range(N - K, N)) + list(range(0, HALF + K))          # 112
ROWS1 = list(range(HALF - K, N)) + list(range(0, K))              # 112

_CACHE = {}
LAST_RESULTS = None          # BassKernelResults of the most recent run


def _kernel_body(tc):
    nc = tc.nc
    # rotated x: row j = node (j-4) mod 208; block0 = rows 0:112,
    # block1 = rows 104:216
    x_d = nc.dram_tensor("xh", [NR, SHARD], BF16, kind="ExternalInput").ap()
    w_d = nc.dram_tensor("wh", [NH, N], BF16, kind="ExternalInput").ap()
    b_d = nc.dram_tensor("bias", [N, 1], FP32, kind="ExternalInput").ap()
    o_d = nc.dram_tensor("outt", [NR, SHARD], BF16, kind="ExternalOutput").ap()

    with ExitStack() as ctx:
        const = ctx.enter_context(tc.tile_pool(name="const", bufs=1))

        # One-time setup: host-premasked halo-ordered weights, one DMA on
        # the Scalar HWDGE ring (idle at startup); bias halves on GpSimd.
        wh = const.tile([NH, N], BF16, tag="wh")
        nc.scalar.dma_start(wh, w_d)
        bA = const.tile([HALF, 1], FP32, tag="bA")
        bB = const.tile([HALF, 1], FP32, tag="bB")
        nc.gpsimd.dma_start(bA, b_d[0:HALF, :])
        nc.gpsimd.dma_start(bB, b_d[HALF:N, :])

        x0p = ctx.enter_context(tc.tile_pool(name="x0p", bufs=3))
        x1p = ctx.enter_context(tc.tile_pool(name="x1p", bufs=3))
        o0p = ctx.enter_context(tc.tile_pool(name="o0p", bufs=3))
        o1p = ctx.enter_context(tc.tile_pool(name="o1p", bufs=3))
        ps0p = ctx.enter_context(tc.tile_pool(name="ps0p", bufs=2, space="PSUM"))
        ps1p = ctx.enter_context(tc.tile_pool(name="ps1p", bufs=2, space="PSUM"))

        for c in range(N_CHUNKS):
            tsl = slice(c * TOUT, (c + 1) * TOUT)
            xh0 = x0p.tile([NH, TOUT], BF16, tag="xh0")
            xh1 = x1p.tile([NH, TOUT], BF16, tag="xh1")
            if c == 0:
                # stores haven't started: split loads across both rings
                nc.sync.dma_start(xh0[0:64, :], x_d[0:64, tsl])
                nc.scalar.dma_start(xh0[64:NH, :], x_d[64:NH, tsl])
                nc.sync.dma_start(xh1[0:64, :], x_d[HALF : HALF + 64, tsl])
                nc.scalar.dma_start(xh1[64:NH, :], x_d[HALF + 64 : NR, tsl])
            else:
                nc.sync.dma_start(xh0, x_d[0:NH, tsl])
                nc.sync.dma_start(xh1, x_d[HALF:NR, tsl])

            o0_t = o0p.tile([NH, TOUT], BF16, tag="o0")
            o1_t = o1p.tile([NH, TOUT], BF16, tag="o1")
            for s in range(SUB):
                g = slice(s * TB2, (s + 1) * TB2)
                ga = slice(s * TB2, s * TB2 + TB)
                gb = slice(s * TB2 + TB, (s + 1) * TB2)
                # [104, 1024] PSUM tiles (2 banks); each matmul fills one bank
                ps0 = ps0p.tile([HALF, TB2], FP32, tag="ps0")
                nc.tensor.matmul(ps0[:, 0:TB], wh[:, 0:HALF], xh0[:, ga], start=True, stop=True)
                nc.tensor.matmul(ps0[:, TB:TB2], wh[:, 0:HALF], xh0[:, gb], start=True, stop=True)
                ps1 = ps1p.tile([HALF, TB2], FP32, tag="ps1")
                nc.tensor.matmul(ps1[:, 0:TB], wh[:, HALF:N], xh1[:, ga], start=True, stop=True)
                nc.tensor.matmul(ps1[:, TB:TB2], wh[:, HALF:N], xh1[:, gb], start=True, stop=True)
                # evictions split across engines: block0 on VectorE,
                # block1 on ScalarE; both fuse the bias and fp32->bf16
                nc.vector.tensor_scalar_add(o0_t[0:HALF, g], ps0, bA)
                nc.scalar.activation(o1_t[0:HALF, g], ps1, IDENT, bias=bB)
            # overlapping stores, same Scalar queue: block0 rows 0:112 first,
            # block1 rows 104:216 second so its valid rows 104..111 win
            nc.scalar.dma_start(o_d[0:NH, tsl], o0_t)
            nc.scalar.dma_start(o_d[HALF:NR, tsl], o1_t)


def _build():
    nc = bacc.Bacc(
        "TRN2",
        target_bir_lowering=False,
        debug=False,
        num_devices=N_CORES,
    )
    with tile.TileContext(nc) as tc:
        _kernel_body(tc)
    nc.compile()
    return nc


def kernel(x, W, b, S):
    global LAST_RESULTS
    nc = _CACHE.get("nc")
    if nc is None:
        nc = _build()
        _CACHE["nc"] = nc

    xf = np.asarray(x, np.float32).reshape(ROWS_TOTAL, N)
    SW = (np.asarray(S, np.float32) * np.asarray(W, np.float32))
    wh = np.empty((NH, N), NP_BF16)
    wh[:, 0:HALF] = SW[ROWS0, 0:HALF]
    wh[:, HALF:N] = SW[ROWS1, HALF:N]
    bf = np.ascontiguousarray(np.asarray(b, np.float32).reshape(N, 1))

    in_maps = []
    for i in range(N_CORES):
        xt = xf[i * SHARD : (i + 1) * SHARD].T          # [208, SHARD] view
        xh = np.empty((NR, SHARD), NP_BF16)
        xh[0:K] = xt[N - K : N]
        xh[K : N + K] = xt
        xh[N + K : NR] = xt[0:K]
        in_maps.append({"xh": xh, "wh": wh, "bias": bf})
    res = run_bass_kernel_spmd(nc, in_maps, core_ids=list(range(N_CORES)))
    LAST_RESULTS = res
    out = np.empty((ROWS_TOTAL, N), np.float32)
    for i, r in enumerate(res.results):
        yt = r["outt"]                                  # [216, SHARD] bf16
        out[i * SHARD : (i + 1) * SHARD, 0:HALF] = yt[0:HALF].T
        out[i * SHARD : (i + 1) * SHARD, HALF:N] = yt[HALF:N].T
    return out.reshape(B, T, N)
